# revision 11
# baseline (speedup 1.0000x reference)
"""
AC_FocalLoss Trainium2 kernel (8 NeuronCores, data-parallel over batch).

Full inputs: input/target [64, 1, 1024, 1024] f32.  Output: scalar f32.

Math (validated vs reference to 1e-16 in f64):
  s  = sigmoid(x); lp = ln(s) = log_sigmoid(x); nlp = -lp
  loss_e = t*nlp - 2*t*s*nlp + s^2*nlp + s^2*x - t*s^2*x      (sum = S_loss)
  aw_e   = s^2 + t - 2*t*s                                     (sum = S_aw)
  thr_b  = mn_b + 0.8*(mx_b - mn_b);  NH = sum_b count(x_b > thr_b)
  result = S_loss / S_aw * (1 - NH/N)^1.5

Per core (8 batches of 1024x1024), per 2048-col quarter-slab:
  ACT    : s = Sigmoid(x) [bf16], t_bf = Copy(t) [bf16, +sum t],
           s2 = Square(s) [bf16, +sum s2]  (all in sigmoid table set),
           lp = Ln(s) [bf16]               (natural_log set)
  DVE    : xn = -x cast [bf16], ts = t*s, ts2 = ts*s,
           per-batch running TT-max/TT-min over xn (exact min/max),
           count compare xn < thrn
  GPSIMD : tiny cross-partition max reduces + threshold scalar math
  PE     : 7 trace accumulations in PSUM (trace(M) = the wanted sum):
           D1=(lp,t) D2=(lp,ts) D4=(lp,s2) D5=(xn,s2) D6=(xn,ts2)
           STS=(ones,ts) CNT=(ones,cmp)
           + per-batch scalar->column broadcast matmuls for thrn
  Host   : trace() of PSUM dumps, final assembly in float64.

Toolchain notes (measured in this container):
  - walrus allows ONE sync wait per instruction -> split_multi_waits
    post-pass hoists extras into NoOps.
  - custom-ISA ops (tensor_tensor_reduce, partition_broadcast, pool, ...)
    fail codegen -> only native mybir instructions.
  - DVE accum_out forces the op to 1x rate -> sums ride ACT accum_out
    (free) or PE ones-matmuls instead.
  - GPSIMD streaming ops are ~10-30x slower than DVE here -> gpsimd only
    does tiny cross-partition reduces.
"""

import os
import sys
import numpy as np
from contextlib import ExitStack

for _p in ("/opt/trn_rl_repo", "/root/.axon_site/_ro/trn_rl_repo"):
    if os.path.isdir(_p) and _p not in sys.path:
        sys.path.insert(0, _p)

from concourse import bass, tile, mybir  # noqa: E402
from concourse.bass_utils import run_bass_kernel_spmd  # noqa: E402

P = 128
B_PER_CORE = 8
H = W = 1024
FD = 2048                      # quarter-slab free dim
QPB = 4                        # quarters per batch (4*2048 = 8*1024 rows)
NQ = B_PER_CORE * QPB          # 32 quarter-slabs per core
NCHUNK = FD // P               # 16 [128,128] chunks per quarter
N_CORES = 8

F32 = mybir.dt.float32
BF16 = mybir.dt.bfloat16
AF = mybir.ActivationFunctionType
ALU = mybir.AluOpType
AX = mybir.AxisListType

# out layout: [128, OUTW] f32
_D_NAMES = ["D1", "D2", "D4", "D5", "D6", "STS", "CNT"]
COL_D = 0                      # 7 * 128 cols of PSUM trace dumps
COL_S2S = COL_D + 7 * P        # [128, NQ] sum s^2 per quarter (ACT accum)
COL_TSUM = COL_S2S + NQ        # [128, NQ] sum t per quarter (ACT accum)
COL_A = COL_TSUM + NQ          # [1, 8] partition 0: A_b = max(-x) = -mn
COL_B = COL_A + B_PER_CORE     # [1, 8]: B_b = max(x) = mx
OUTW = COL_B + B_PER_CORE


def split_multi_waits(nc):
    """walrus codegen only supports 1 sync wait per instruction; hoist
    extra waits into preceding single-wait NoOps on the same engine."""
    cnt = 0
    for f in nc.m.functions:
        for bb in f.blocks:
            new_list = []
            for ins in bb.instructions:
                si = ins.sync_info
                if si is not None and si.on_wait and len(si.on_wait) > 1:
                    waits = list(si.on_wait)
                    for w in waits[:-1]:
                        nop = mybir.InstNoOp(name=f"nopw{cnt}", ins=[], outs=[])
                        cnt += 1
                        nop.engine = ins.engine
                        nop.sync_info = mybir.SyncInfo(on_wait=[w], on_update=[])
                        new_list.append(nop)
                    si.on_wait = [waits[-1]]
                new_list.append(ins)
            bb.instructions[:] = new_list
    return cnt


def build_nc():
    nc = bass.Bass("TRN2", target_bir_lowering=False, debug=False)
    x_ext = nc.declare_dram_parameter("input", [B_PER_CORE, H, W], F32, isOutput=False)
    t_ext = nc.declare_dram_parameter("target", [B_PER_CORE, H, W], F32, isOutput=False)
    out_ext = nc.declare_dram_parameter("out", [P, OUTW], F32, isOutput=True)

    x_r = x_ext.ap().rearrange("b (g p) w -> b p g w", p=P)   # [8, 128, 8, 1024]
    t_r = t_ext.ap().rearrange("b (g p) w -> b p g w", p=P)

    with tile.TileContext(nc) as tc, ExitStack() as ctx:
        xp = ctx.enter_context(tc.tile_pool(name="xp", bufs=4))
        tp = ctx.enter_context(tc.tile_pool(name="tp", bufs=3))
        sbfp = ctx.enter_context(tc.tile_pool(name="sbfp", bufs=4))
        lpp = ctx.enter_context(tc.tile_pool(name="lpp", bufs=4))
        xnp = ctx.enter_context(tc.tile_pool(name="xnp", bufs=6))
        tbfp = ctx.enter_context(tc.tile_pool(name="tbfp", bufs=3))
        tsp = ctx.enter_context(tc.tile_pool(name="tsp", bufs=3))
        ts2p = ctx.enter_context(tc.tile_pool(name="ts2p", bufs=3))
        s2p = ctx.enter_context(tc.tile_pool(name="s2p", bufs=3))
        junkp = ctx.enter_context(tc.tile_pool(name="junkp", bufs=2))
        mmp = ctx.enter_context(tc.tile_pool(name="mmp", bufs=2))
        accp = ctx.enter_context(tc.tile_pool(name="accp", bufs=1))
        psum = ctx.enter_context(tc.tile_pool(name="psum", bufs=1, space="PSUM"))

        D = {nm: psum.tile([P, P], F32, name=f"psum_{nm}") for nm in _D_NAMES}
        thrps = psum.tile([P, B_PER_CORE], F32, name="thrps")

        outbuf = accp.tile([P, OUTW], F32)
        s2s = outbuf[:, COL_S2S:COL_S2S + NQ]
        tsum = outbuf[:, COL_TSUM:COL_TSUM + NQ]

        ones1 = accp.tile([1, P], F32)
        nc.vector.memset(ones1[:], 1.0)
        ones_bf = accp.tile([P, P], BF16)
        nc.vector.memset(ones_bf[:], 1.0)
        negmn_p = accp.tile([P, B_PER_CORE], F32)   # per-partition max(-x)
        mx_p = accp.tile([P, B_PER_CORE], F32)      # per-partition max(x)
        scalA = accp.tile([1, B_PER_CORE], F32)
        scalB = accp.tile([1, B_PER_CORE], F32)
        thrn_sc = accp.tile([1, B_PER_CORE], F32)
        tmp_sc = accp.tile([1, B_PER_CORE], F32)
        thr_sb = accp.tile([P, B_PER_CORE], F32)

        first_mm = [True]

        for b in range(B_PER_CORE):
            x_tiles = {}
            t_tiles = {}
            xn_tiles = {}
            s_tiles = {}
            lp_tiles = {}
            t_bf_tiles = {}
            s2_tiles = {}
            accmax = mmp.tile([P, FD], BF16, name=f"accmax_{b}", tag="accmax")
            accmin = mmp.tile([P, FD], BF16, name=f"accmin_{b}", tag="accmin")

            for qq in range(QPB):
                x = xp.tile([P, FD], F32)
                nc.sync.dma_start(
                    out=x[:].rearrange("p (g w) -> p g w", g=2),
                    in_=x_r[b][:, 2 * qq:2 * qq + 2, :],
                )
                x_tiles[qq] = x
                t = tp.tile([P, FD], F32)
                nc.sync.dma_start(
                    out=t[:].rearrange("p (g w) -> p g w", g=2),
                    in_=t_r[b][:, 2 * qq:2 * qq + 2, :],
                )
                t_tiles[qq] = t

            # ACT phase 1 (sigmoid set): sigmoid, copy-cast t (+sum t),
            # square (+sum s2)
            for qq in range(QPB):
                s_bf = sbfp.tile([P, FD], BF16)
                nc.scalar.activation(s_bf[:], x_tiles[qq][:], AF.Sigmoid)
                s_tiles[qq] = s_bf
            for qq in range(QPB):
                qi = b * QPB + qq
                t_bf = tbfp.tile([P, FD], BF16)
                nc.scalar.activation(t_bf[:], t_tiles[qq][:], AF.Copy,
                                     accum_out=tsum[:, qi:qi + 1])
                t_bf_tiles[qq] = t_bf
            for qq in range(QPB):
                qi = b * QPB + qq
                s2 = s2p.tile([P, FD], BF16)
                nc.scalar.activation(s2[:], s_tiles[qq][:], AF.Square,
                                     accum_out=s2s[:, qi:qi + 1])
                s2_tiles[qq] = s2
            # ACT phase 2 (natural_log set): lp = ln(s)
            for qq in range(QPB):
                lp = lpp.tile([P, FD], BF16)
                nc.scalar.activation(lp[:], s_tiles[qq][:], AF.Ln)
                lp_tiles[qq] = lp

            # DVE: xn cast, products, running min/max; PE: trace dots
            for qq in range(QPB):
                qi = b * QPB + qq
                s_bf = s_tiles[qq]
                t_bf = t_bf_tiles[qq]
                s2 = s2_tiles[qq]
                xn = xnp.tile([P, FD], BF16)
                nc.vector.tensor_scalar(xn[:], x_tiles[qq][:], -1.0, None, ALU.mult)
                xn_tiles[qq] = xn

                ts = tsp.tile([P, FD], BF16)
                nc.vector.tensor_mul(ts[:], t_bf[:], s_bf[:])
                ts2 = ts2p.tile([P, FD], BF16)
                nc.vector.tensor_mul(ts2[:], ts[:], s_bf[:])

                if qq == 0:
                    nc.vector.tensor_copy(accmax[:], xn[:])
                    nc.vector.tensor_copy(accmin[:], xn[:])
                else:
                    nc.vector.tensor_tensor(accmax[:], accmax[:], xn[:], ALU.max)
                    nc.vector.tensor_tensor(accmin[:], accmin[:], xn[:], ALU.min)

                lp = lp_tiles[qq]
                for k in range(NCHUNK):
                    sl = slice(k * P, (k + 1) * P)
                    st = first_mm[0]
                    first_mm[0] = False
                    last = (qi == NQ - 1) and (k == NCHUNK - 1)
                    nc.tensor.matmul(D["D1"][:], lp[:, sl], t_bf[:, sl], start=st, stop=last)
                    nc.tensor.matmul(D["D2"][:], lp[:, sl], ts[:, sl], start=st, stop=last)
                    nc.tensor.matmul(D["D4"][:], lp[:, sl], s2[:, sl], start=st, stop=last)
                    nc.tensor.matmul(D["D5"][:], xn[:, sl], s2[:, sl], start=st, stop=last)
                    nc.tensor.matmul(D["D6"][:], xn[:, sl], ts2[:, sl], start=st, stop=last)
                    nc.tensor.matmul(D["STS"][:], ones_bf[:], ts[:, sl], start=st, stop=last)

            # min/max stage 2: free-dim reduce, negate min-side, then
            # cross-partition max on gpsimd (only add/avg/max supported)
            nc.vector.tensor_reduce(negmn_p[:, b:b + 1], accmax[:], AX.X, ALU.max)
            nc.vector.tensor_reduce(mx_p[:, b:b + 1], accmin[:], AX.X, ALU.min)
            nc.vector.tensor_scalar(mx_p[:, b:b + 1], mx_p[:, b:b + 1], -1.0, None, ALU.mult)
            nc.gpsimd.tensor_reduce(scalA[0:1, b:b + 1], negmn_p[:, b:b + 1], AX.C, ALU.max)
            nc.gpsimd.tensor_reduce(scalB[0:1, b:b + 1], mx_p[:, b:b + 1], AX.C, ALU.max)
            # thrn = -thr = 0.2*A - 0.8*B
            nc.gpsimd.tensor_scalar(tmp_sc[0:1, b:b + 1], scalB[0:1, b:b + 1], -0.8, None, ALU.mult)
            nc.gpsimd.tensor_scalar(thrn_sc[0:1, b:b + 1], scalA[0:1, b:b + 1], 0.2, None, ALU.mult)
            nc.gpsimd.tensor_tensor(thrn_sc[0:1, b:b + 1], thrn_sc[0:1, b:b + 1],
                                    tmp_sc[0:1, b:b + 1], ALU.add)
            # broadcast scalar -> [128,1] via ones-matmul, copy PSUM->SBUF
            nc.tensor.matmul(thrps[:, b:b + 1], ones1[0:1, :], thrn_sc[0:1, b:b + 1],
                             start=True, stop=True)
            nc.vector.tensor_copy(thr_sb[:, b:b + 1], thrps[:, b:b + 1])
            # debug outputs
            nc.gpsimd.tensor_copy(outbuf[0:1, COL_A + b:COL_A + b + 1], scalA[0:1, b:b + 1])
            nc.gpsimd.tensor_copy(outbuf[0:1, COL_B + b:COL_B + b + 1], scalB[0:1, b:b + 1])

            # count sweep: count(x > thr) == count(xn < thrn); count via PE
            for qq in range(QPB):
                qi = b * QPB + qq
                junk = junkp.tile([P, FD], BF16)
                nc.vector.tensor_scalar(junk[:], xn_tiles[qq][:], thr_sb[:, b:b + 1], None,
                                        ALU.is_lt)
                for k in range(NCHUNK):
                    sl = slice(k * P, (k + 1) * P)
                    last = (qi == NQ - 1) and (k == NCHUNK - 1)
                    nc.tensor.matmul(D["CNT"][:], ones_bf[:], junk[:, sl],
                                     start=(qi == 0 and k == 0), stop=last)

        # finalize: copy PSUM accumulators into outbuf, single DMA out
        for i, nm in enumerate(_D_NAMES):
            nc.vector.tensor_copy(outbuf[:, COL_D + i * P:COL_D + (i + 1) * P], D[nm][:])
        nc.sync.dma_start(out=out_ext.ap()[:], in_=outbuf[:])

    split_multi_waits(nc)
    return nc


_NC_CACHE = None


def _get_nc():
    global _NC_CACHE
    if _NC_CACHE is None:
        _NC_CACHE = build_nc()
    return _NC_CACHE


def _execute(x, t, trace=False):
    """x, t: [64, 1, 1024, 1024] f32 numpy.  Returns (scalar, exec_time_ns)."""
    x = np.ascontiguousarray(x.reshape(64, H, W), dtype=np.float32)
    t = np.ascontiguousarray(t.reshape(64, H, W), dtype=np.float32)
    in_maps = []
    for c in range(N_CORES):
        sl = slice(c * B_PER_CORE, (c + 1) * B_PER_CORE)
        in_maps.append({"input": x[sl], "target": t[sl]})

    nc = _get_nc()
    res = run_bass_kernel_spmd(nc, in_maps, list(range(N_CORES)), trace=trace)

    D1 = D2 = D4 = D5 = D6 = 0.0
    D7 = D8 = D9 = 0.0
    NH = 0.0
    for c in range(N_CORES):
        o = res.results[c]["out"].astype(np.float64)   # [128, OUTW]
        tr = [np.trace(o[:, COL_D + i * P:COL_D + (i + 1) * P]) for i in range(7)]
        D1 += tr[0]; D2 += tr[1]; D4 += tr[2]; D5 += tr[3]; D6 += tr[4]
        D9 += tr[5]; NH += tr[6]
        D7 += o[:, COL_S2S:COL_S2S + NQ].sum()
        D8 += o[:, COL_TSUM:COL_TSUM + NQ].sum()

    # device dots: D1,D2,D4 against lp = -nlp; D5,D6 against xn = -x
    S_loss = -D1 + 2.0 * D2 - D4 - D5 + D6
    S_aw = D7 + D8 - 2.0 * D9
    n_total = float(x.size)
    val = S_loss / S_aw * (1.0 - NH / n_total) ** 1.5
    return np.float32(val), res.exec_time_ns


def kernel(input, target):
    val, _ = _execute(np.asarray(input), np.asarray(target))
    return val


if __name__ == "__main__":
    rng = np.random.default_rng(0)
    x = rng.standard_normal((64, 1, H, W)).astype(np.float32)
    t = rng.random((64, 1, H, W)).astype(np.float32)
    val, ns = _execute(x, t, trace=True)
    print("kernel:", val, "exec_ns:", ns)
